# revision 15
# baseline (speedup 1.0000x reference)
"""MoE (top-2 of 8 experts) Trainium2 kernel.

Strategy: expert-parallel across 8 NeuronCores (1 expert per core).
Each core (SPMD, same NEFF, different inputs):
  1. Router: logits = x @ Wr computed on-device (replicated on every core).
  2. Softmax (fp32) + top-2 via DVE max8/max_index.
  3. Build this expert's combine weight + selection mask per token.
  4. Compact selected tokens: exclusive prefix-sum over the token mask
     (strict-lower-triangular matmul along partitions + DVE scan along the
     tile axis), then indirect-DMA scatter of [x_row | weight | token_id]
     records into a dense DRAM buffer (capacity C).
  5. Gather records back, PE-transpose x-rows into [H, C] layout.
  6. Expert MLP on the <=C compacted tokens:
     hT = gelu_tanh(W1^T xg^T + b1), y = hT^T W2 (+b2), y *= weight.
     fp32 matmuls, PSUM accumulation, partial sums accumulated into the
     output DRAM via SWDGE accumulate-DMA.
  7. Host scatters each core's compact [C, H] output back to token order
     (indices are a device output) and sums the (disjoint-slot) updates.

Self-contained: shapes hardcoded for x[2,2048,1024], E=8, FF=4096, K=2.
"""

import sys

sys.path.insert(0, "/opt/trn_rl_repo")

import numpy as np

import concourse.bass as bass
import concourse.mybir as mybir
import concourse.tile as tile
from concourse import bacc
from concourse.bass import IndirectOffsetOnAxis
from concourse.bass_utils import run_bass_kernel_spmd
from concourse.masks import make_identity

F32 = mybir.dt.float32
I32 = mybir.dt.int32
U32 = mybir.dt.uint32

# Problem sizes (hardcoded for the graded problem).
FULL_CFG = dict(T=4096, H=1024, FF=4096, E=8, C=1280)
BIG = 1.0e6  # out-of-range slot for unselected tokens (dropped by bounds check)


def build_moe_bass(cfg):
    """Build + compile the single-core SPMD module."""
    nc = _build_moe_body(cfg)
    nc.compile()
    return nc


def _build_moe_body(cfg):
    """Trace the kernel into a Bacc module."""
    T, H, FF, E, C = cfg["T"], cfg["H"], cfg["FF"], cfg["E"], cfg["C"]
    TT = T // 128          # token tiles
    HK = H // 128          # hidden (contraction) chunks
    FFK = FF // 128        # ff tiles
    GF = min(8, FFK)       # ff tiles per super-block
    NSB = FFK // GF        # number of ff super-blocks
    CT = C // 128          # compact token tiles
    XGW = H + 2            # scatter record width: x row + weight + token id
    RB = min(512, T)       # router token block (columns of xT per load)
    PHASES = cfg.get("phases", 5)
    ABLK = 512             # W1 psum block (<= 1 PSUM bank of fp32)

    nc = bacc.Bacc("TRN2", target_bir_lowering=False, debug=False, num_devices=8)

    # ---- I/O ----
    xT_d = nc.dram_tensor("xt_in", [H, T], F32, kind="ExternalInput").ap()
    x_d = nc.dram_tensor("x_in", [T, H], F32, kind="ExternalInput").ap()
    wr_d = nc.dram_tensor("wr_in", [128, HK * E], F32, kind="ExternalInput").ap()
    w1_d = nc.dram_tensor("w1_in", [H, FF], F32, kind="ExternalInput").ap()
    w2_d = nc.dram_tensor("w2_in", [FF, H], F32, kind="ExternalInput").ap()
    b1_d = nc.dram_tensor("b1_in", [128, FFK], F32, kind="ExternalInput").ap()
    b2_d = nc.dram_tensor("b2_in", [128, H], F32, kind="ExternalInput").ap()
    eid_d = nc.dram_tensor("eid_in", [128, 1], F32, kind="ExternalInput").ap()
    tid_d = nc.dram_tensor("tid_in", [128, TT], F32, kind="ExternalInput").ap()
    tri_d = nc.dram_tensor("tri_in", [128, 128], F32, kind="ExternalInput").ap()
    e0_d = nc.dram_tensor("e0_in", [128, 128], F32, kind="ExternalInput").ap()
    ones_d = nc.dram_tensor("ones_in", [128, 128], F32, kind="ExternalInput").ap()

    yc_d = nc.dram_tensor("yc_out", [C, H], F32, kind="ExternalOutput").ap()
    lg_d = nc.dram_tensor("lg_out", [128, TT * E], F32, kind="ExternalOutput").ap()
    sel_d = nc.dram_tensor("sel_out", [128, TT * 2], I32, kind="ExternalOutput").ap()
    idx_d = nc.dram_tensor("idx_out", [128, CT], I32, kind="ExternalOutput").ap()
    # Scatter target; ExternalOutput so the runner pre-zeroes it on device.
    xg_d = nc.dram_tensor("xg_scratch", [C, XGW], F32, kind="ExternalOutput").ap()

    with tile.TileContext(nc) as tc:
        with (
            tc.tile_pool(name="const", bufs=1) as cp,
            tc.tile_pool(name="persist", bufs=1) as pp,
        ):
            # ---- constants ----
            wr_sb = cp.tile([128, HK * E], F32)
            nc.sync.dma_start(out=wr_sb[:], in_=wr_d[:, :])
            b1_sb = cp.tile([128, FFK], F32)
            nc.sync.dma_start(out=b1_sb[:], in_=b1_d[:, :])
            b2_sb = cp.tile([128, H], F32)
            nc.sync.dma_start(out=b2_sb[:], in_=b2_d[:, :])
            eid_sb = cp.tile([128, 1], F32)
            nc.sync.dma_start(out=eid_sb[:], in_=eid_d[:, :])
            tid_sb = cp.tile([128, TT], F32)
            nc.sync.dma_start(out=tid_sb[:], in_=tid_d[:, :])
            tri_sb = cp.tile([128, 128], F32)
            nc.sync.dma_start(out=tri_sb[:], in_=tri_d[:, :])
            e0_sb = cp.tile([128, 128], F32)
            nc.sync.dma_start(out=e0_sb[:], in_=e0_d[:, :])
            ones_sb = cp.tile([128, 128], F32)
            nc.sync.dma_start(out=ones_sb[:], in_=ones_d[:, :])
            ident = cp.tile([128, 128], F32)
            make_identity(nc, ident[:])

            # ---- persistent intermediates ----
            logits_all = pp.tile([128, TT * E], F32)

            # ======== Phase 1: router logits (tokens on partitions) ========
            with (
                tc.tile_pool(name="xt", bufs=2) as xtp,
                tc.tile_pool(name="rps", bufs=4, space="PSUM") as rps,
            ):
                for tb in range(T // RB):
                    xts = []
                    for hk in range(HK):
                        t_ = xtp.tile([128, RB], F32, tag=f"xt{hk}", name=f"xt{hk}")
                        nc.sync.dma_start(
                            out=t_[:],
                            in_=xT_d[hk * 128 : (hk + 1) * 128, tb * RB : (tb + 1) * RB],
                        )
                        xts.append(t_)
                    for j4 in range(RB // 128):
                        jt = tb * (RB // 128) + j4
                        ps = rps.tile([128, E], F32, name="rlog")
                        for hk in range(HK):
                            nc.tensor.matmul(
                                ps[:],
                                lhsT=xts[hk][:, j4 * 128 : (j4 + 1) * 128],
                                rhs=wr_sb[:, hk * E : (hk + 1) * E],
                                start=(hk == 0),
                                stop=(hk == HK - 1),
                            )
                        nc.vector.tensor_copy(
                            out=logits_all[:, jt * E : (jt + 1) * E], in_=ps[:]
                        )
            nc.sync.dma_start(out=lg_d[:, :], in_=logits_all[:])

            # ======== Phase 2: softmax + top-2 + combine + slot ========
            if PHASES < 2:
                return nc
            wt_tok = pp.tile([128, TT], F32)   # this expert's combine weight / token
            slot_i = pp.tile([128, TT], I32)   # compact slot per token (BIG if unused)
            with (
                tc.tile_pool(name="sm", bufs=1) as smp,
                tc.tile_pool(name="cps", bufs=1, space="PSUM") as cps,
            ):
                l3 = logits_all[:].rearrange("p (t e) -> p t e", e=E)
                rmax = smp.tile([128, TT], F32)
                nc.vector.tensor_reduce(
                    out=rmax[:], in_=l3, axis=mybir.AxisListType.X, op=mybir.AluOpType.max
                )
                shifted = smp.tile([128, TT * E], F32)
                nc.vector.tensor_tensor(
                    out=shifted[:].rearrange("p (t e) -> p t e", e=E),
                    in0=l3,
                    in1=rmax[:, :, None].to_broadcast([128, TT, E]),
                    op=mybir.AluOpType.subtract,
                )
                probs = smp.tile([128, TT * E], F32)
                nc.scalar.activation(
                    out=probs[:], in_=shifted[:], func=mybir.ActivationFunctionType.Exp
                )
                p3 = probs[:].rearrange("p (t e) -> p t e", e=E)
                rsum = smp.tile([128, TT], F32)
                nc.vector.tensor_reduce(
                    out=rsum[:], in_=p3, axis=mybir.AxisListType.X, op=mybir.AluOpType.add
                )
                rinv = smp.tile([128, TT], F32)
                nc.vector.reciprocal(out=rinv[:], in_=rsum[:])
                nc.vector.tensor_tensor(
                    out=p3,
                    in0=p3,
                    in1=rinv[:, :, None].to_broadcast([128, TT, E]),
                    op=mybir.AluOpType.mult,
                )

                maxs = smp.tile([128, TT * 8], F32)
                idxs = smp.tile([128, TT * 8], U32)
                for j in range(TT):
                    nc.vector.max(
                        out=maxs[:, j * 8 : (j + 1) * 8],
                        in_=probs[:, j * E : (j + 1) * E],
                    )
                    nc.vector.max_index(
                        out=idxs[:, j * 8 : (j + 1) * 8],
                        in_max=maxs[:, j * 8 : (j + 1) * 8],
                        in_values=probs[:, j * E : (j + 1) * E],
                    )
                m3 = maxs[:].rearrange("p (t e) -> p t e", e=8)
                i3 = idxs[:].rearrange("p (t e) -> p t e", e=8)

                sel_i32 = smp.tile([128, TT * 2], I32)
                nc.vector.tensor_copy(
                    out=sel_i32[:].rearrange("p (t k) -> p t k", k=2),
                    in_=i3[:, :, 0:2],
                )
                nc.sync.dma_start(out=sel_d[:, :], in_=sel_i32[:])

                i1f = smp.tile([128, TT], F32)
                i2f = smp.tile([128, TT], F32)
                nc.vector.tensor_copy(out=i1f[:, :, None], in_=i3[:, :, 0:1])
                nc.vector.tensor_copy(out=i2f[:, :, None], in_=i3[:, :, 1:2])
                eq1 = smp.tile([128, TT], F32)
                eq2 = smp.tile([128, TT], F32)
                nc.vector.tensor_scalar(
                    out=eq1[:], in0=i1f[:], scalar1=eid_sb[:, 0:1], scalar2=None,
                    op0=mybir.AluOpType.is_equal,
                )
                nc.vector.tensor_scalar(
                    out=eq2[:], in0=i2f[:], scalar1=eid_sb[:, 0:1], scalar2=None,
                    op0=mybir.AluOpType.is_equal,
                )
                w1c = smp.tile([128, TT], F32)
                w2c = smp.tile([128, TT], F32)
                nc.vector.tensor_copy(out=w1c[:, :, None], in_=m3[:, :, 0:1])
                nc.vector.tensor_copy(out=w2c[:, :, None], in_=m3[:, :, 1:2])
                t1 = smp.tile([128, TT], F32)
                t2 = smp.tile([128, TT], F32)
                nc.vector.tensor_tensor(out=t1[:], in0=eq1[:], in1=w1c[:], op=mybir.AluOpType.mult)
                nc.vector.tensor_tensor(out=t2[:], in0=eq2[:], in1=w2c[:], op=mybir.AluOpType.mult)
                nc.vector.tensor_tensor(out=wt_tok[:], in0=t1[:], in1=t2[:], op=mybir.AluOpType.add)
                mask_tok = smp.tile([128, TT], F32)
                nc.vector.tensor_tensor(out=mask_tok[:], in0=eq1[:], in1=eq2[:], op=mybir.AluOpType.add)

                # -- exclusive prefix sum over tokens (order: j*128 + p) --
                pos_ps = cps.tile([128, TT], F32, space="PSUM", name="pos_ps")
                nc.tensor.matmul(
                    pos_ps[:], lhsT=tri_sb[:], rhs=mask_tok[:], start=True, stop=False
                )
                # column sums (broadcast to all partitions via all-ones matmul)
                colsum_ps = cps.tile([128, TT], F32, space="PSUM", name="colsum_ps")
                nc.tensor.matmul(
                    colsum_ps[:], lhsT=ones_sb[:], rhs=mask_tok[:], start=True, stop=True
                )
                # row-0 scan -> exclusive column offsets
                cs_t = smp.tile([128, TT], F32)
                off_t = smp.tile([128, TT], F32)
                nc.vector.memset(off_t[:], 0.0)
                nc.vector.tensor_copy(out=cs_t[0:1, :], in_=colsum_ps[0:1, :])
                incl_t = smp.tile([128, TT], F32)
                nc.vector.tensor_tensor_scan(
                    out=incl_t[0:1, :], data0=cs_t[0:1, :],
                    data1=cs_t[0:1, :], initial=0.0,
                    op0=mybir.AluOpType.add, op1=mybir.AluOpType.bypass,
                )
                nc.vector.tensor_tensor(
                    out=off_t[0:1, :], in0=incl_t[0:1, :],
                    in1=cs_t[0:1, :], op=mybir.AluOpType.subtract,
                )
                nc.tensor.matmul(
                    pos_ps[:], lhsT=e0_sb[:], rhs=off_t[:], start=False, stop=True
                )
                # slot = mask ? pos : BIG
                stmp = smp.tile([128, TT], F32)
                nc.vector.tensor_scalar(
                    out=stmp[:], in0=mask_tok[:], scalar1=-BIG, scalar2=BIG,
                    op0=mybir.AluOpType.mult, op1=mybir.AluOpType.add,
                )
                slot_f = smp.tile([128, TT], F32)
                nc.vector.tensor_tensor(
                    out=slot_f[:], in0=pos_ps[:], in1=stmp[:], op=mybir.AluOpType.add
                )
                nc.vector.tensor_copy(out=slot_i[:], in_=slot_f[:])

            # ======== Phase 3: scatter [x | wt | tid] records to xg ========
            if PHASES < 3:
                return nc
            with tc.tile_pool(name="stg", bufs=3) as stgp:
                for j in range(TT):
                    stg = stgp.tile([128, XGW], F32, tag="stg", name="stg")
                    nc.sync.dma_start(
                        out=stg[:, 0:H], in_=x_d[j * 128 : (j + 1) * 128, :]
                    )
                    nc.vector.tensor_copy(out=stg[:, H : H + 1], in_=wt_tok[:, j : j + 1])
                    nc.vector.tensor_copy(out=stg[:, H + 1 : H + 2], in_=tid_sb[:, j : j + 1])
                    nc.gpsimd.indirect_dma_start(
                        out=xg_d[:, :],
                        out_offset=IndirectOffsetOnAxis(ap=slot_i[:, j : j + 1], axis=0),
                        in_=stg[:],
                        in_offset=None,
                        bounds_check=C - 1,
                        oob_is_err=False,
                    )

            # ======== Phase 4: gather back + transpose to [H, C] ========
            if PHASES < 4:
                return nc
            xgT = [pp.tile([128, C], F32, tag=f"xgT{hk}", name=f"xgT{hk}") for hk in range(HK)]
            wt_slot = pp.tile([128, CT], F32)
            with (
                tc.tile_pool(name="gb", bufs=3) as gbp,
                tc.tile_pool(name="tps", bufs=4, space="PSUM") as tps,
            ):
                idxf = pp.tile([128, CT], F32)
                for ct in range(CT):
                    g = gbp.tile([128, XGW], F32, tag="gb", name="gb")
                    nc.sync.dma_start(out=g[:], in_=xg_d[ct * 128 : (ct + 1) * 128, :])
                    nc.vector.tensor_copy(out=wt_slot[:, ct : ct + 1], in_=g[:, H : H + 1])
                    nc.vector.tensor_copy(out=idxf[:, ct : ct + 1], in_=g[:, H + 1 : H + 2])
                    for hk in range(HK):
                        pt = tps.tile([128, 128], F32, space="PSUM", name="tpt")
                        nc.tensor.transpose(
                            pt[:], g[:, hk * 128 : (hk + 1) * 128], ident[:]
                        )
                        nc.vector.tensor_copy(
                            out=xgT[hk][:, ct * 128 : (ct + 1) * 128], in_=pt[:]
                        )
                idx_i = pp.tile([128, CT], I32)
                nc.vector.tensor_copy(out=idx_i[:], in_=idxf[:])
                nc.sync.dma_start(out=idx_d[:, :], in_=idx_i[:])

            # ======== Phase 5: expert MLP over compacted tokens ========
            if PHASES < 5:
                return nc
            with (
                tc.tile_pool(name="wgt", bufs=1) as wp,
                tc.tile_pool(name="hts", bufs=1) as hp,
                tc.tile_pool(name="stage", bufs=3) as sp,
                tc.tile_pool(name="pA", bufs=2, space="PSUM") as pA,
                tc.tile_pool(name="pY", bufs=2, space="PSUM") as pY,
            ):
                for sb in range(NSB):
                    w1t = []
                    for hk in range(HK):
                        t_ = wp.tile([128, GF * 128], F32, tag=f"w1_{hk}", name=f"w1_{hk}")
                        nc.sync.dma_start(
                            out=t_[:],
                            in_=w1_d[
                                hk * 128 : (hk + 1) * 128,
                                sb * GF * 128 : (sb + 1) * GF * 128,
                            ],
                        )
                        w1t.append(t_)
                    w2t = []
                    for k8 in range(GF):
                        t_ = wp.tile([128, H], F32, tag=f"w2_{k8}", name=f"w2_{k8}")
                        nc.sync.dma_start(
                            out=t_[:],
                            in_=w2_d[
                                (sb * GF + k8) * 128 : (sb * GF + k8 + 1) * 128, :
                            ],
                        )
                        w2t.append(t_)

                    # --- W1 + gelu: hT[ffk] = gelu(W1_chunk^T @ xgT + b1) ---
                    hts = []
                    for k8 in range(GF):
                        ffk = sb * GF + k8
                        h_sb = hp.tile([128, C], F32, tag=f"ht{k8}", name=f"ht{k8}")
                        for a0 in range(0, C, ABLK):
                            aw = min(ABLK, C - a0)
                            pa = pA.tile([128, ABLK], F32, space="PSUM", tag="pa", name="pa")
                            for hk in range(HK):
                                nc.tensor.matmul(
                                    pa[:, 0:aw],
                                    lhsT=w1t[hk][:, k8 * 128 : (k8 + 1) * 128],
                                    rhs=xgT[hk][:, a0 : a0 + aw],
                                    start=(hk == 0),
                                    stop=(hk == HK - 1),
                                )
                            if cfg.get("gelu", "hw") == "hw":
                                nc.scalar.activation(
                                    out=h_sb[:, a0 : a0 + aw],
                                    in_=pa[:, 0:aw],
                                    func=mybir.ActivationFunctionType.Gelu_apprx_tanh,
                                    bias=b1_sb[:, ffk : ffk + 1],
                                )
                            else:
                                # composite tanh gelu (CoreSim lacks Gelu)
                                u = sp.tile([128, ABLK], F32, tag="gl_u", name="gl_u")
                                nc.scalar.activation(
                                    out=u[:, 0:aw], in_=pa[:, 0:aw],
                                    func=mybir.ActivationFunctionType.Identity,
                                    bias=b1_sb[:, ffk : ffk + 1],
                                )
                                v = sp.tile([128, ABLK], F32, tag="gl_v", name="gl_v")
                                nc.vector.tensor_tensor(
                                    out=v[:, 0:aw], in0=u[:, 0:aw], in1=u[:, 0:aw],
                                    op=mybir.AluOpType.mult,
                                )
                                nc.vector.tensor_tensor(
                                    out=v[:, 0:aw], in0=v[:, 0:aw], in1=u[:, 0:aw],
                                    op=mybir.AluOpType.mult,
                                )
                                nc.vector.tensor_scalar(
                                    out=v[:, 0:aw], in0=v[:, 0:aw],
                                    scalar1=0.044715, scalar2=None,
                                    op0=mybir.AluOpType.mult,
                                )
                                nc.vector.tensor_tensor(
                                    out=v[:, 0:aw], in0=v[:, 0:aw], in1=u[:, 0:aw],
                                    op=mybir.AluOpType.add,
                                )
                                nc.scalar.activation(
                                    out=v[:, 0:aw], in_=v[:, 0:aw],
                                    func=mybir.ActivationFunctionType.Tanh,
                                    scale=float(np.sqrt(2.0 / np.pi)),
                                )
                                nc.vector.tensor_scalar(
                                    out=v[:, 0:aw], in0=v[:, 0:aw],
                                    scalar1=1.0, scalar2=0.5,
                                    op0=mybir.AluOpType.add,
                                    op1=mybir.AluOpType.mult,
                                )
                                nc.vector.tensor_tensor(
                                    out=h_sb[:, a0 : a0 + aw], in0=v[:, 0:aw],
                                    in1=u[:, 0:aw], op=mybir.AluOpType.mult,
                                )
                        hts.append(h_sb)

                    # --- W2: y[tt] += hT^T @ W2_chunk, weighted, to DRAM ---
                    for tt in range(CT):
                        py = pY.tile([128, H], F32, space="PSUM", tag="py", name="py")
                        for k8 in range(GF):
                            for n0 in range(0, H, 512):
                                nw = min(512, H - n0)
                                nc.tensor.matmul(
                                    py[:, n0 : n0 + nw],
                                    lhsT=hts[k8][:, tt * 128 : (tt + 1) * 128],
                                    rhs=w2t[k8][:, n0 : n0 + nw],
                                    start=(k8 == 0),
                                    stop=(k8 == GF - 1),
                                )
                        stage = sp.tile([128, H], F32, tag="stage", name="stage")
                        if sb == 0:
                            nc.vector.tensor_tensor(
                                out=stage[:], in0=py[:], in1=b2_sb[:],
                                op=mybir.AluOpType.add,
                            )
                            nc.vector.tensor_scalar(
                                out=stage[:], in0=stage[:],
                                scalar1=wt_slot[:, tt : tt + 1], scalar2=None,
                                op0=mybir.AluOpType.mult,
                            )
                        else:
                            nc.vector.tensor_scalar(
                                out=stage[:], in0=py[:],
                                scalar1=wt_slot[:, tt : tt + 1], scalar2=None,
                                op0=mybir.AluOpType.mult,
                            )
                        nc.gpsimd.dma_start(
                            out=yc_d[tt * 128 : (tt + 1) * 128, :],
                            in_=stage[:],
                            accum_op=mybir.AluOpType.add,
                        )
    return nc


def host_inputs_for_core(e, xs, Wr, W1, b1, W2, b2, cfg):
    """Per-core input dict (host-side sharding + layout prep)."""
    T, H, FF, E, C = cfg["T"], cfg["H"], cfg["FF"], cfg["E"], cfg["C"]
    TT, HK, FFK, CT = T // 128, H // 128, FF // 128, C // 128
    f = np.float32
    tid = (np.arange(TT)[None, :] * 128 + np.arange(128)[:, None]).astype(f)
    tri = np.triu(np.ones((128, 128), f), 1)  # tri[k,p] = 1 if k < p
    e0 = np.zeros((128, 128), f)
    e0[0, :] = 1.0
    return {
        "xt_in": np.ascontiguousarray(xs.T),
        "x_in": np.ascontiguousarray(xs),
        "wr_in": np.ascontiguousarray(
            Wr.reshape(HK, 128, E).transpose(1, 0, 2).reshape(128, HK * E)
        ),
        "w1_in": np.ascontiguousarray(W1[e]),
        "w2_in": np.ascontiguousarray(W2[e]),
        "b1_in": np.ascontiguousarray(b1[e].reshape(FFK, 128).T),
        "b2_in": np.ascontiguousarray(np.tile(b2[e][None, :], (128, 1))),
        "eid_in": np.full((128, 1), float(e), f),
        "tid_in": tid,
        "tri_in": tri,
        "e0_in": e0,
        "ones_in": np.ones((128, 128), f),
    }


_NC_CACHE = {}
LAST_RESULT = None  # BassKernelResults of the most recent kernel() call


def _get_nc(cfg_key):
    if cfg_key not in _NC_CACHE:
        _NC_CACHE[cfg_key] = build_moe_bass(FULL_CFG)
    return _NC_CACHE[cfg_key]


def kernel(x, Wr, W1, b1, W2, b2):
    cfg = FULL_CFG
    T, H, E, C = cfg["T"], cfg["H"], cfg["E"], cfg["C"]
    TT, CT = T // 128, C // 128

    x = np.asarray(x, np.float32)
    Wr = np.asarray(Wr, np.float32)
    W1 = np.asarray(W1, np.float32)
    b1 = np.asarray(b1, np.float32)
    W2 = np.asarray(W2, np.float32)
    b2 = np.asarray(b2, np.float32)
    xs = x.reshape(T, H)

    in_maps = [host_inputs_for_core(e, xs, Wr, W1, b1, W2, b2, cfg) for e in range(E)]
    nc = _get_nc("full")
    res = run_bass_kernel_spmd(nc, in_maps, core_ids=list(range(E)))
    global LAST_RESULT
    LAST_RESULT = res
    results = res.results

    # Router outputs (identical on every core; take core 0).
    lg = results[0]["lg_out"]
    logits = lg.reshape(128, TT, E).transpose(1, 0, 2).reshape(T, E)
    sel = results[0]["sel_out"].reshape(128, TT, 2).transpose(1, 0, 2).reshape(T, 2)

    out = np.zeros((T, H), np.float32)
    for e in range(E):
        n_e = int((sel == e).sum())
        if n_e == 0:
            continue
        idx = results[e]["idx_out"].T.reshape(C)[:n_e]
        yc = results[e]["yc_out"][:n_e]
        out[idx] += yc
    return (
        out.reshape(x.shape),
        logits.astype(np.float32),
        sel.astype(np.int32),
    )


if __name__ == "__main__":
    import time

    t0 = time.time()
    nc = build_moe_bass(FULL_CFG)
    print(f"built ok in {time.time() - t0:.1f}s")
